# revision 1
# baseline (speedup 1.0000x reference)
"""Trainium2 Bass kernel for nn_Decoder (show-attend-tell style LSTM decoder).

Strategy: data-parallel over batch (32 seqs/core x 8 cores), zero collectives.
 - Phase 1: att1 = enc @ W_enc'.T  (|W_full|-scaled), written to DRAM scratch.
 - Phase 2: h0/c0 from host-computed mean_enc (bias via ones-row K-extension).
 - Phase 3: E_pre[t] = emb_seq_t @ W_ih_emb.T + (b_ih+b_hh)  (batched over t).
 - Loop t=0..19 (fully unrolled): attention scores via DVE bcast-add + relu
   (sign/|W| folding turns W_full@relu into sign-weighted PE dot), softmax,
   awe via per-batch PE matvecs with fp8 enc weights (mixed fp8xbf16 matmul),
   f_beta gate, LSTM gates (W_ih enc-part streamed from HBM each step),
   pointwise LSTM + ragged-length masking, h_new stored to DRAM.
 - Phase 5: deferred vocab projection [640,512]@[512,10000] (M=128-efficient).

All activations/weights bf16 (fp32 accumulation in PSUM), enc in fp8-e4m3 for
the attention-weighted-encoding (attention is near-uniform -> quantization
noise averages out), cell state c in fp32.
"""
import numpy as np
import ml_dtypes
from contextlib import ExitStack

import concourse.bass as bass
import concourse.tile as tile
from concourse import bacc, mybir
from concourse.bass_utils import run_bass_kernel_spmd

F32 = mybir.dt.float32
BF16 = mybir.dt.bfloat16
FP8 = mybir.dt.float8e4
NP_BF16 = ml_dtypes.bfloat16
NP_FP8 = ml_dtypes.float8_e4m3

AX = mybir.AxisListType
OP = mybir.AluOpType
AF = mybir.ActivationFunctionType

B, P, ENC, EMB, DEC, ATT, V, L = 256, 196, 2048, 512, 512, 512, 10000, 21
T = L - 1
NCORES = 8
BL = B // NCORES          # 32 seqs per core
BP = BL * P               # 6272
EC = ENC // 128           # 16
AT = ATT // 128           # 4
DC = DEC // 128           # 4
GT = (4 * DEC) // 128     # 16
G = 4 * DEC               # 2048
VT = 500                  # vocab N-tile
NBT = (T * BL) // 128     # 5 bt-tiles of 128 (4 t x 32 b)
CH = 4                    # batches per score chunk
CHW = CH * P              # 784

_BUILT = None


def _build():
    nc = bacc.Bacc("TRN2", target_bir_lowering=False, debug=False,
                   num_devices=NCORES)

    # ---- external inputs (per-core payload; weights identical across cores)
    encT_in = nc.dram_tensor("encT", [EC, 128, BP], BF16, kind="ExternalInput").ap()
    encS_in = nc.dram_tensor("encS", [4, 128, 2, BL // 4, EC, 128], BF16, kind="ExternalInput").ap()
    wencT_in = nc.dram_tensor("wencT", [128, EC, ATT], BF16, kind="ExternalInput").ap()
    bencp_in = nc.dram_tensor("bencp", [128, AT], F32, kind="ExternalInput").ap()
    wdecT_in = nc.dram_tensor("wdecT", [128, 5, ATT], BF16, kind="ExternalInput").ap()
    wfbT_in = nc.dram_tensor("wfbT", [EC, 128, 5, 128], BF16, kind="ExternalInput").ap()
    whhT_in = nc.dram_tensor("whhT", [128, 4, G], BF16, kind="ExternalInput").ap()
    wihencT_in = nc.dram_tensor("wihencT", [EC, 128, G], BF16, kind="ExternalInput").ap()
    wihembT_in = nc.dram_tensor("wihembT", [128, 5, G], BF16, kind="ExternalInput").ap()
    embT_in = nc.dram_tensor("embT", [128, 5, T * BL], BF16, kind="ExternalInput").ap()
    winitT_in = nc.dram_tensor("winitT", [128, 17, 2, DEC], BF16, kind="ExternalInput").ap()
    meanT_in = nc.dram_tensor("meanT", [128, 17, BL], BF16, kind="ExternalInput").ap()
    wfcT_in = nc.dram_tensor("wfcT", [DC, 128, V], BF16, kind="ExternalInput").ap()
    sign_in = nc.dram_tensor("signw", [128, AT], BF16, kind="ExternalInput").ap()
    mask_in = nc.dram_tensor("mask", [T, 128, BL], F32, kind="ExternalInput").ap()
    identf_in = nc.dram_tensor("identf", [128, 128], F32, kind="ExternalInput").ap()
    identb_in = nc.dram_tensor("identb", [128, 128], BF16, kind="ExternalInput").ap()

    pred_out = nc.dram_tensor("pred", [NBT, 128, V], F32, kind="ExternalOutput").ap()

    # ---- internal DRAM scratch
    att1_dram = nc.dram_tensor("att1_scr", [AT, 128, BP], BF16).ap()
    epre_dram = nc.dram_tensor("epre_scr", [T, GT, 128, BL], BF16).ap()
    hhist_dram = nc.dram_tensor("hhist_scr", [T, DC, 128, BL], BF16).ap()

    with tile.TileContext(nc) as tc:
        with ExitStack() as octx:
            cpool = octx.enter_context(tc.tile_pool(name="const", bufs=1))

            wdecT_sb = cpool.tile([128, 5, ATT], BF16)
            nc.sync.dma_start(wdecT_sb[:], wdecT_in[:])
            whhT_sb = cpool.tile([128, 4, G], BF16)
            nc.sync.dma_start(whhT_sb[:], whhT_in[:])
            sign_sb = cpool.tile([128, AT], BF16)
            nc.sync.dma_start(sign_sb[:], sign_in[:])
            bencp_sb = cpool.tile([128, AT], F32)
            nc.sync.dma_start(bencp_sb[:], bencp_in[:])
            identf_sb = cpool.tile([128, 128], F32)
            nc.sync.dma_start(identf_sb[:], identf_in[:])
            identb_sb = cpool.tile([128, 128], BF16)
            nc.sync.dma_start(identb_sb[:], identb_in[:])

            h_sb = cpool.tile([128, 5, BL], BF16)       # 4 d-chunks + const [1;0]
            c_sb = cpool.tile([128, 4, BL], F32)
            alphaT_sb = cpool.tile([128, 2, BL], BF16)  # p-chunks; rows>=68 of pc1 stay 0
            nc.gpsimd.memset(alphaT_sb[:], 0.0)
            nc.gpsimd.memset(h_sb[:, 4, :], 0.0)
            nc.gpsimd.memset(h_sb[0:1, 4, :], 1.0)

            # ================= P1: att1 =================
            with ExitStack() as ctx:
                wp = ctx.enter_context(tc.tile_pool(name="p1w", bufs=1))
                pp = ctx.enter_context(tc.tile_pool(name="p1in", bufs=2))
                op_ = ctx.enter_context(tc.tile_pool(name="p1out", bufs=3))
                ps1 = ctx.enter_context(tc.tile_pool(name="p1ps", bufs=3, space=bass.MemorySpace.PSUM))

                wencT_sb = wp.tile([128, EC, ATT], BF16)
                nc.sync.dma_start(wencT_sb[:], wencT_in[:])
                NPAN = 16
                PAN = BP // NPAN  # 392
                for pan in range(NPAN):
                    et = pp.tile([128, EC, PAN], BF16)
                    nc.sync.dma_start(
                        et[:], encT_in[:, :, pan * PAN:(pan + 1) * PAN]
                        .rearrange("ec p j -> p ec j"))
                    for at in range(AT):
                        ps = ps1.tile([128, PAN], F32)
                        for ec in range(EC):
                            nc.tensor.matmul(
                                ps[:],
                                wencT_sb[:, ec, at * 128:(at + 1) * 128],
                                et[:, ec, :],
                                start=(ec == 0), stop=(ec == EC - 1))
                        ob = op_.tile([128, PAN], BF16)
                        nc.vector.tensor_scalar(
                            ob[:], ps[:], bencp_sb[:, at:at + 1], None, OP.add)
                        nc.sync.dma_start(
                            att1_dram[at, :, pan * PAN:(pan + 1) * PAN], ob[:])

            # ================= P2: h0 / c0 =================
            with ExitStack() as ctx:
                wp = ctx.enter_context(tc.tile_pool(name="p2w", bufs=1))
                ps2 = ctx.enter_context(tc.tile_pool(name="p2ps", bufs=2, space=bass.MemorySpace.PSUM))
                winit_sb = wp.tile([128, 17, 2, DEC], BF16)
                nc.sync.dma_start(winit_sb[:], winitT_in[:])
                mean_sb = wp.tile([128, 17, BL], BF16)
                nc.sync.dma_start(mean_sb[:], meanT_in[:])
                for hc in range(2):
                    for dc in range(DC):
                        ps = ps2.tile([128, BL], F32)
                        for ek in range(17):
                            nc.tensor.matmul(
                                ps[:], winit_sb[:, ek, hc, dc * 128:(dc + 1) * 128],
                                mean_sb[:, ek, :],
                                start=(ek == 0), stop=(ek == 16))
                        if hc == 0:
                            nc.vector.tensor_copy(h_sb[:, dc, :], ps[:])
                        else:
                            nc.vector.tensor_copy(c_sb[:, dc, :], ps[:])

            # ================= P3: E_pre =================
            with ExitStack() as ctx:
                wp = ctx.enter_context(tc.tile_pool(name="p3w", bufs=1))
                op_ = ctx.enter_context(tc.tile_pool(name="p3o", bufs=3))
                ps3 = ctx.enter_context(tc.tile_pool(name="p3ps", bufs=3, space=bass.MemorySpace.PSUM))
                wihe_sb = wp.tile([128, 5, G], BF16)
                nc.sync.dma_start(wihe_sb[:], wihembT_in[:])
                embT_sb = wp.tile([128, 5, T * BL], BF16)
                nc.sync.dma_start(embT_sb[:], embT_in[:])
                HT = (T * BL) // 2  # 320
                for gt in range(GT):
                    for ns in range(2):
                        ps = ps3.tile([128, HT], F32)
                        for ek in range(5):
                            nc.tensor.matmul(
                                ps[:], wihe_sb[:, ek, gt * 128:(gt + 1) * 128],
                                embT_sb[:, ek, ns * HT:(ns + 1) * HT],
                                start=(ek == 0), stop=(ek == 4))
                        ob = op_.tile([128, HT], BF16)
                        nc.vector.tensor_copy(ob[:], ps[:])
                        nc.sync.dma_start(
                            epre_dram[ns * (T // 2):(ns + 1) * (T // 2), gt]
                            .rearrange("t g b -> g t b"),
                            ob[:].rearrange("g (t b) -> g t b", b=BL))

            # ================= recurrent loop =================
            with ExitStack() as ctx:
                a1p = ctx.enter_context(tc.tile_pool(name="a1", bufs=2))
                wkp = ctx.enter_context(tc.tile_pool(name="wk", bufs=2))
                fbp = ctx.enter_context(tc.tile_pool(name="fb", bufs=2))
                smp = ctx.enter_context(tc.tile_pool(name="sm", bufs=2))
                ptp = ctx.enter_context(tc.tile_pool(name="pt", bufs=1))
                encp_pool = ctx.enter_context(tc.tile_pool(name="encp", bufs=2))
                sfp = ctx.enter_context(tc.tile_pool(name="sf", bufs=1))
                pp_att2 = ctx.enter_context(tc.tile_pool(name="psA", bufs=1, space=bass.MemorySpace.PSUM))
                pp_dot = ctx.enter_context(tc.tile_pool(name="psD", bufs=2, space=bass.MemorySpace.PSUM))
                pp_tp = ctx.enter_context(tc.tile_pool(name="psT", bufs=1, space=bass.MemorySpace.PSUM))
                pp_big = ctx.enter_context(tc.tile_pool(name="psB", bufs=1, space=bass.MemorySpace.PSUM))

                for t in range(T):
                    mask_t = smp.tile([128, BL], F32, tag="mask")
                    nc.sync.dma_start(mask_t[:], mask_in[t])
                    ep_t = smp.tile([128, GT, BL], BF16, tag="ep")
                    nc.sync.dma_start(ep_t[:], epre_dram[t].rearrange("gt g b -> g gt b"))

                    # --- att2' = W_dec' @ h + b_dec'
                    att2_ps = pp_att2.tile([128, AT * BL], F32)
                    for at in range(AT):
                        for dk in range(5):
                            nc.tensor.matmul(
                                att2_ps[:, at * BL:(at + 1) * BL],
                                wdecT_sb[:, dk, at * 128:(at + 1) * 128],
                                h_sb[:, dk, :],
                                start=(dk == 0), stop=(dk == 4))
                    att2_sb = smp.tile([128, AT, BL], BF16, tag="att2")
                    nc.vector.tensor_copy(
                        att2_sb[:].rearrange("p a b -> p (a b)"), att2_ps[:])

                    # --- scores: z = relu(att1' + att2'), s = sign . z
                    scores_fl = sfp.tile([1, BP], BF16, tag="scoresf")
                    NSUB = [512, 272]  # 784 split (psum bank limit)
                    for c in range(BL // CH):
                        a1 = a1p.tile([128, AT, CHW], BF16)
                        nc.sync.dma_start(
                            a1[:], att1_dram[:, :, c * CHW:(c + 1) * CHW]
                            .rearrange("a p j -> p a j"))
                        for at in range(AT):
                            nc.vector.tensor_tensor(
                                a1[:, at, :].rearrange("p (b q) -> p b q", q=P),
                                a1[:, at, :].rearrange("p (b q) -> p b q", q=P),
                                att2_sb[:, at, c * CH:(c + 1) * CH]
                                .rearrange("p (b u) -> p b u", u=1)
                                .broadcast_to([128, CH, P]),
                                OP.add)
                        nc.vector.tensor_scalar(
                            a1[:].rearrange("p a j -> p (a j)"),
                            a1[:].rearrange("p a j -> p (a j)"),
                            0.0, None, OP.max)
                        off = 0
                        for si, nsub in enumerate(NSUB):
                            ps = pp_dot.tile([1, 512], F32)
                            for at in range(AT):
                                nc.tensor.matmul(
                                    ps[:, 0:nsub], sign_sb[:, at:at + 1],
                                    a1[:, at, off:off + nsub],
                                    start=(at == 0), stop=(at == AT - 1))
                            eng = nc.scalar if si % 2 == 0 else nc.vector
                            if eng is nc.scalar:
                                eng.activation(
                                    scores_fl[:, c * CHW + off:c * CHW + off + nsub],
                                    ps[:, 0:nsub], AF.Copy)
                            else:
                                eng.tensor_copy(
                                    scores_fl[:, c * CHW + off:c * CHW + off + nsub],
                                    ps[:, 0:nsub])
                            off += nsub
                    scores_bp = smp.tile([BL, P], BF16, tag="scores")
                    nc.sync.dma_start(
                        scores_bp[:],
                        scores_fl[:].rearrange("u (b q) -> u b q", q=P))

                    # --- softmax over p
                    mx = smp.tile([BL, 1], F32, tag="mx")
                    nc.vector.tensor_reduce(mx[:], scores_bp[:], AX.XYZW, OP.max,
                                            negate=True)
                    ex = smp.tile([BL, P], F32, tag="ex")
                    sume = smp.tile([BL, 1], F32, tag="sume")
                    nc.scalar.activation(ex[:], scores_bp[:], AF.Exp, bias=mx[:],
                                         accum_out=sume[:])
                    rc = smp.tile([BL, 1], F32, tag="rc")
                    nc.vector.reciprocal(rc[:], sume[:])
                    al = smp.tile([BL, P], F32, tag="al")
                    nc.vector.tensor_scalar(al[:], ex[:], rc[:], None, OP.mult)

                    # --- transpose alpha -> [p, b] bf16 (padded rows stay 0)
                    tp = pp_tp.tile([128, 2 * BL], F32)
                    nc.tensor.transpose(tp[:, 0:BL], al[:, 0:128], identf_sb[0:BL, 0:BL])
                    nc.tensor.transpose(tp[0:P - 128, BL:2 * BL], al[:, 128:P],
                                        identf_sb[0:BL, 0:BL])
                    nc.vector.tensor_copy(alphaT_sb[:, 0, :], tp[:, 0:BL])
                    nc.vector.tensor_copy(alphaT_sb[0:P - 128, 1, :],
                                          tp[0:P - 128, BL:2 * BL])

                    # --- awe (bf16 enc weights streamed x bf16 alpha)
                    awe_ps = pp_big.tile([128, EC * BL], F32, tag="awe")
                    BG = BL // 4  # 8 batches per streamed group
                    for bg in range(4):
                        encw = encp_pool.tile([128, 2, BG, EC, 128], BF16)
                        nc.sync.dma_start(encw[:], encS_in[bg])
                        for b8 in range(BG):
                            b = bg * BG + b8
                            for et in range(EC):
                                for pc in range(2):
                                    nc.tensor.matmul(
                                        awe_ps[:, et * BL + b:et * BL + b + 1],
                                        encw[:, pc, b8, et, :],
                                        alphaT_sb[:, pc, b:b + 1],
                                        start=(pc == 0), stop=(pc == 1))

                    # --- f_beta gate
                    gb_ps = pp_big.tile([128, EC * BL], F32, tag="gb")
                    for et in range(EC):
                        fbt = fbp.tile([128, 5, 128], BF16)
                        nc.sync.dma_start(fbt[:], wfbT_in[et])
                        for dk in range(5):
                            nc.tensor.matmul(
                                gb_ps[:, et * BL:(et + 1) * BL],
                                fbt[:, dk, :], h_sb[:, dk, :],
                                start=(dk == 0), stop=(dk == 4))
                    gate_s = ptp.tile([128, EC * BL], BF16, tag="gate")
                    nc.scalar.activation(gate_s[:], gb_ps[:], AF.Sigmoid)
                    xenc = ptp.tile([128, EC, BL], BF16, tag="xenc")
                    nc.vector.tensor_tensor(
                        xenc[:].rearrange("p e b -> p (e b)"), gate_s[:],
                        awe_ps[:], OP.mult)

                    # --- gates = W_ihenc @ xenc + W_hh @ h + E_pre
                    g_ps = pp_big.tile([128, GT * BL], F32, tag="gps")
                    for k in range(EC):
                        wk = wkp.tile([128, G], BF16)
                        nc.sync.dma_start(wk[:], wihencT_in[k])
                        for gt in range(GT):
                            nc.tensor.matmul(
                                g_ps[:, gt * BL:(gt + 1) * BL],
                                wk[:, gt * 128:(gt + 1) * 128],
                                xenc[:, k, :],
                                start=(k == 0 and gt == 0), stop=False)
                    for dk in range(4):
                        for gt in range(GT):
                            nc.tensor.matmul(
                                g_ps[:, gt * BL:(gt + 1) * BL],
                                whhT_sb[:, dk, gt * 128:(gt + 1) * 128],
                                h_sb[:, dk, :],
                                start=False, stop=False)
                    for gt in range(GT):
                        nc.tensor.matmul(
                            g_ps[:, gt * BL:(gt + 1) * BL],
                            identb_sb[:], ep_t[:, gt, :],
                            start=False, stop=(gt == GT - 1))

                    # --- pointwise LSTM (layout [128, 4*BL] per gate)
                    W4 = 4 * BL  # 128
                    sig_if = ptp.tile([128, 2 * W4], F32, tag="sif")
                    nc.scalar.activation(sig_if[:], g_ps[:, 0:2 * W4], AF.Sigmoid)
                    tanh_g = ptp.tile([128, W4], F32, tag="tg")
                    nc.scalar.activation(tanh_g[:], g_ps[:, 2 * W4:3 * W4], AF.Tanh)
                    sig_o = ptp.tile([128, W4], F32, tag="so")
                    nc.scalar.activation(sig_o[:], g_ps[:, 3 * W4:4 * W4], AF.Sigmoid)

                    cflat = c_sb[:].rearrange("p k b -> p (k b)")
                    fc_ = ptp.tile([128, W4], F32, tag="fc")
                    nc.vector.tensor_tensor(fc_[:], sig_if[:, W4:2 * W4], cflat, OP.mult)
                    ig_ = ptp.tile([128, W4], F32, tag="ig")
                    nc.vector.tensor_tensor(ig_[:], sig_if[:, 0:W4], tanh_g[:], OP.mult)
                    c_new = ptp.tile([128, W4], F32, tag="cn")
                    nc.vector.tensor_tensor(c_new[:], fc_[:], ig_[:], OP.add)
                    tanh_c = ptp.tile([128, W4], F32, tag="tc")
                    nc.scalar.activation(tanh_c[:], c_new[:], AF.Tanh)
                    h_new = ptp.tile([128, W4], F32, tag="hn")
                    nc.vector.tensor_tensor(h_new[:], sig_o[:], tanh_c[:], OP.mult)

                    mb = mask_t[:].rearrange("p (u b) -> p u b", u=1) \
                        .broadcast_to([128, 4, BL])
                    def v3(ap):
                        return ap.rearrange("p (k b) -> p k b", b=BL)
                    # c carry: c += m * (c_new - c)
                    dlt = ptp.tile([128, W4], F32, tag="dlt")
                    nc.vector.tensor_tensor(dlt[:], c_new[:], cflat, OP.subtract)
                    nc.vector.tensor_tensor(v3(dlt[:]), v3(dlt[:]), mb, OP.mult)
                    nc.vector.tensor_tensor(cflat, cflat, dlt[:], OP.add)
                    # h store (masked h_new)
                    h_st = ptp.tile([128, W4], BF16, tag="hst")
                    nc.vector.tensor_tensor(v3(h_st[:]), v3(h_new[:]), mb, OP.mult)
                    nc.sync.dma_start(
                        hhist_dram[t].rearrange("k d b -> d k b"),
                        h_st[:].rearrange("p (k b) -> p k b", b=BL))
                    # h carry: h += m * (h_new - h)
                    hflat = h_sb[:, 0:4, :].rearrange("p k b -> p (k b)")
                    dlh = ptp.tile([128, W4], F32, tag="dlh")
                    nc.vector.tensor_tensor(dlh[:], h_new[:], hflat, OP.subtract)
                    nc.vector.tensor_tensor(v3(dlh[:]), v3(dlh[:]), mb, OP.mult)
                    nc.vector.tensor_tensor(hflat, hflat, dlh[:], OP.add)

            # ================= P5: vocab projection =================
            with ExitStack() as ctx:
                hp = ctx.enter_context(tc.tile_pool(name="p5h", bufs=1))
                wp = ctx.enter_context(tc.tile_pool(name="p5w", bufs=2))
                op_ = ctx.enter_context(tc.tile_pool(name="p5o", bufs=3))
                ps5 = ctx.enter_context(tc.tile_pool(name="p5ps", bufs=4, space=bass.MemorySpace.PSUM))
                hh_sb = hp.tile([128, DC, T, BL], BF16)
                for dc in range(DC):
                    nc.sync.dma_start(
                        hh_sb[:, dc, :, :],
                        hhist_dram[:, dc].rearrange("t d b -> d t b"))
                for vt in range(V // VT):
                    wv = wp.tile([128, DC, VT], BF16)
                    nc.sync.dma_start(
                        wv[:], wfcT_in[:, :, vt * VT:(vt + 1) * VT]
                        .rearrange("k d v -> d k v"))
                    for btc in range(NBT):
                        ps = ps5.tile([128, VT], F32)
                        for dc in range(DC):
                            nc.tensor.matmul(
                                ps[:],
                                hh_sb[:, dc, btc * 4:(btc + 1) * 4, :]
                                .rearrange("d t b -> d (t b)"),
                                wv[:, dc, :],
                                start=(dc == 0), stop=(dc == DC - 1))
                        ob = op_.tile([128, VT], F32)
                        nc.vector.tensor_copy(ob[:], ps[:])
                        nc.sync.dma_start(
                            pred_out[btc, :, vt * VT:(vt + 1) * VT], ob[:])

    nc.compile()
    return nc


def _get_nc():
    global _BUILT
    if _BUILT is None:
        _BUILT = _build()
    return _BUILT


def _prep(inputs):
    enc = np.asarray(inputs["encoder_out"], np.float32)
    caps = np.asarray(inputs["encoded_captions"]).astype(np.int64)
    lens = np.asarray(inputs["caption_lengths"]).astype(np.int64)[:, 0]
    emb = np.asarray(inputs["emb"], np.float32)
    W_enc_att = np.asarray(inputs["W_enc_att"], np.float32)
    b_enc_att = np.asarray(inputs["b_enc_att"], np.float32)
    W_dec_att = np.asarray(inputs["W_dec_att"], np.float32)
    b_dec_att = np.asarray(inputs["b_dec_att"], np.float32)
    W_full_att = np.asarray(inputs["W_full_att"], np.float32)
    W_init_h = np.asarray(inputs["W_init_h"], np.float32)
    b_init_h = np.asarray(inputs["b_init_h"], np.float32)
    W_init_c = np.asarray(inputs["W_init_c"], np.float32)
    b_init_c = np.asarray(inputs["b_init_c"], np.float32)
    W_f_beta = np.asarray(inputs["W_f_beta"], np.float32)
    b_f_beta = np.asarray(inputs["b_f_beta"], np.float32)
    W_fc = np.asarray(inputs["W_fc"], np.float32)
    b_fc = np.asarray(inputs["b_fc"], np.float32)
    W_ih = np.asarray(inputs["W_ih"], np.float32)
    W_hh = np.asarray(inputs["W_hh"], np.float32)
    b_ih = np.asarray(inputs["b_ih"], np.float32)
    b_hh = np.asarray(inputs["b_hh"], np.float32)

    sort_ind = np.argsort(-lens, kind="stable")
    enc_s = enc[sort_ind]
    caps_s = caps[sort_ind]
    dec_len = (lens[sort_ind] - 1).astype(np.int64)
    emb_seq = emb[caps_s[:, :T]]                      # [B, T, EMB]
    mean_enc = enc_s.mean(axis=1)                     # [B, ENC]
    masks = (np.arange(T)[None, :] < dec_len[:, None]).astype(np.float32)

    wf = W_full_att[0]                                # [ATT]
    absw, signw = np.abs(wf), np.sign(wf).astype(np.float32)
    Wenc_p = absw[:, None] * W_enc_att                # [ATT, ENC]
    benc_p = absw * b_enc_att
    Wdec_p = absw[:, None] * W_dec_att
    bdec_p = absw * b_dec_att

    def bf(x):
        return np.ascontiguousarray(x).astype(NP_BF16)

    # ---- shared weight payloads
    wencT = bf(Wenc_p.T.reshape(EC, 128, ATT).transpose(1, 0, 2))
    bencp = np.ascontiguousarray(benc_p.reshape(AT, 128).T).astype(np.float32)

    def kext(WT, bias, kchunks):
        # WT: [K, M] -> [K/128(+1), 128, M] with extra chunk row0 = bias
        Wc = WT.reshape(kchunks, 128, WT.shape[1])
        ext = np.zeros((1, 128, WT.shape[1]), np.float32)
        ext[0, 0, :] = bias
        return np.concatenate([Wc, ext], axis=0)

    wdecT = bf(kext(Wdec_p.T, bdec_p, 4).transpose(1, 0, 2))          # [128,5,ATT]
    wfbT_full = kext(W_f_beta.T, b_f_beta, 4)                         # [5,128,ENC]
    wfbT = bf(wfbT_full.reshape(5, 128, EC, 128).transpose(2, 1, 0, 3))  # [EC,128,5,128]
    whhT = bf(W_hh.T.reshape(4, 128, G).transpose(1, 0, 2))           # [128,4,G]
    wihencT = bf(W_ih[:, EMB:].T.reshape(EC, 128, G))                 # [EC,128,G]
    wihembT = bf(kext(W_ih[:, :EMB].T, b_ih + b_hh, 4).transpose(1, 0, 2))  # [128,5,G]
    winit = np.stack([W_init_h.T, W_init_c.T], axis=1)                # [ENC,2,DEC]
    winitc = winit.reshape(EC, 128, 2, DEC)
    wext = np.zeros((1, 128, 2, DEC), np.float32)
    wext[0, 0, 0, :] = b_init_h
    wext[0, 0, 1, :] = b_init_c
    winitT = bf(np.concatenate([winitc, wext], axis=0).transpose(1, 0, 2, 3))  # [128,17,2,DEC]
    wfcT = bf(W_fc.T.reshape(DC, 128, V))
    signw_t = bf(signw.reshape(AT, 128).T)
    identf = np.eye(128, dtype=np.float32)
    identb = np.eye(128, dtype=np.float32).astype(NP_BF16)

    in_maps = []
    for cidx in range(NCORES):
        sl = slice(cidx * BL, (cidx + 1) * BL)
        e = enc_s[sl]                                             # [BL,P,ENC]
        encT = bf(e.transpose(2, 0, 1).reshape(EC, 128, BP))
        ep = np.zeros((BL, 256, ENC), np.float32)
        ep[:, :P, :] = e
        encS = ep.reshape(4, BL // 4, 2, 128, EC, 128).transpose(0, 3, 2, 1, 4, 5).astype(NP_BF16)
        es = emb_seq[sl].transpose(2, 1, 0).reshape(4, 128, T * BL)  # [4,128,T*BL]
        esx = np.concatenate([es, np.zeros((1, 128, T * BL), np.float32)], axis=0)
        esx[4, 0, :] = 1.0
        embT = bf(esx.transpose(1, 0, 2))                          # [128,5,T*BL]
        mn = mean_enc[sl].T.reshape(EC, 128, BL)
        mnx = np.concatenate([mn, np.zeros((1, 128, BL), np.float32)], axis=0)
        mnx[16, 0, :] = 1.0
        meanT = bf(mnx.transpose(1, 0, 2))                         # [128,17,BL]
        maskr = np.ascontiguousarray(
            np.broadcast_to(masks[sl].T[:, None, :], (T, 128, BL))).astype(np.float32)
        in_maps.append(dict(
            encT=encT, encS=encS, wencT=wencT, bencp=bencp, wdecT=wdecT,
            wfbT=wfbT, whhT=whhT, wihencT=wihencT, wihembT=wihembT,
            embT=embT, winitT=winitT, meanT=meanT, wfcT=wfcT, signw=signw_t,
            mask=maskr, identf=identf, identb=identb))
    return in_maps, masks, b_fc


def kernel(**inputs):
    nc = _get_nc()
    in_maps, masks, b_fc = _prep(inputs)
    res = run_bass_kernel_spmd(nc, in_maps, core_ids=list(range(NCORES)))
    preds = []
    for cidx in range(NCORES):
        p = res.results[cidx]["pred"]                  # [NBT,128,V]
        p = p.reshape(NBT, 4, BL, V).reshape(T, BL, V).transpose(1, 0, 2)
        preds.append(p)
    out = np.concatenate(preds, axis=0)                # [B,T,V] (sorted order)
    if np.any(b_fc):
        out += masks[:, :, None] * b_fc[None, None, :]
    return np.ascontiguousarray(out, dtype=np.float32)

